# revision 1
# baseline (speedup 1.0000x reference)
"""Data-parallel Trainium2 kernel for nn_A3Cmicropolis10x10linAC.

Strategy (per sharding hint): pure data parallelism. The batch dim (4096)
of x/hx/cx is sharded across the 8 NeuronCores (512 samples each); all
weights (<2MB total) are replicated. Each core runs the full
conv_in -> 5x ConvLSTM -> conv_out -> MLP head pipeline on its shard;
outputs are concatenated on host. Shapes are hardcoded from the problem
spec; this file is self-contained.
"""

import numpy as np
import jax
import jax.numpy as jnp

B, C_IN, H, W = 4096, 14, 10, 10
NH = 16
NCORES = 8
BL = B // NCORES  # 512 samples per core
CELL_ORDER = (2, 1, 0, 3, 4)

WEIGHT_NAMES = (
    "conv_in_w", "conv_in_b", "gates_w", "gates_b", "conv_out_w",
    "conv_out_b", "w0", "b0", "w1", "b1", "wc", "bc", "wa", "ba",
)


def _conv2d(x, w, b, pad):
    y = jax.lax.conv_general_dilated(
        x, w, window_strides=(1, 1), padding=[(pad, pad), (pad, pad)],
        dimension_numbers=("NCHW", "OIHW", "NCHW"))
    return y + b[None, :, None, None]


def _cell(x, h, c, w, b):
    gates = _conv2d(jnp.concatenate([x, h], axis=1), w, b, pad=1)
    i, r, o, g = jnp.split(gates, 4, axis=1)
    i, r, o = jax.nn.sigmoid(i), jax.nn.sigmoid(r), jax.nn.sigmoid(o)
    g = jnp.tanh(g)
    c_new = r * c + i * g
    h_new = o * jnp.tanh(c_new)
    return h_new, c_new


def _forward(x, hx, cx, conv_in_w, conv_in_b, gates_w, gates_b,
             conv_out_w, conv_out_b, w0, b0, w1, b1, wc, bc, wa, ba):
    h = jax.nn.relu(_conv2d(x, conv_in_w, conv_in_b, pad=0))
    hs, cs = [], []
    cur = h
    for k in range(5):
        idx = CELL_ORDER[k]
        cur, c_new = _cell(cur, hx[idx], cx[idx], gates_w[k], gates_b[k])
        hs.append(cur)
        cs.append(c_new)
    y = jnp.tanh(_conv2d(cur, conv_out_w, conv_out_b, pad=1))
    y = y.reshape(y.shape[0], -1)
    y = jnp.tanh(y @ w0.T + b0)
    y = jnp.tanh(y @ w1.T + b1)
    value = y @ wc.T + bc
    action = y @ wa.T + ba
    return value, action, jnp.stack(hs), jnp.stack(cs)


_PMAPPED = jax.pmap(
    _forward,
    axis_name="core",
    in_axes=(0, 0, 0) + (None,) * len(WEIGHT_NAMES),
    devices=jax.devices()[:NCORES],
)


def kernel(**inputs):
    x = np.asarray(inputs["x"])
    hx = np.asarray(inputs["hx"])
    cx = np.asarray(inputs["cx"])

    # Shard batch across cores: device axis leading.
    xs = x.reshape(NCORES, BL, C_IN, H, W)
    hxs = np.ascontiguousarray(
        hx.reshape(5, NCORES, BL, NH, H, W).transpose(1, 0, 2, 3, 4, 5))
    cxs = np.ascontiguousarray(
        cx.reshape(5, NCORES, BL, NH, H, W).transpose(1, 0, 2, 3, 4, 5))

    weights = [np.asarray(inputs[n]) for n in WEIGHT_NAMES]

    value, action, hs, cs = _PMAPPED(xs, hxs, cxs, *weights)

    value = np.asarray(value).reshape(B, 1)
    action = np.asarray(action).reshape(B, 800)
    hs = np.asarray(hs).transpose(1, 0, 2, 3, 4, 5).reshape(5, B, NH, H, W)
    cs = np.asarray(cs).transpose(1, 0, 2, 3, 4, 5).reshape(5, B, NH, H, W)
    return (value.astype(np.float32), action.astype(np.float32),
            hs.astype(np.float32), cs.astype(np.float32))


# revision 2
# speedup vs baseline: 1.1405x; 1.1405x over previous
"""Data-parallel Trainium2 kernel for nn_A3Cmicropolis10x10linAC.

Strategy (per sharding hint): pure data parallelism. The batch dim (4096)
of x/hx/cx is sharded across the 8 NeuronCores (512 samples each); all
weights (<2MB total) are replicated. Each core runs the full
conv_in -> 5x ConvLSTM -> conv_out -> MLP head pipeline on its shard;
outputs are concatenated on host. Shapes are hardcoded from the problem
spec; this file is self-contained.
"""

import numpy as np
import jax
import jax.numpy as jnp

B, C_IN, H, W = 4096, 14, 10, 10
NH = 16
NCORES = 8
BL = B // NCORES  # 512 samples per core
CELL_ORDER = (2, 1, 0, 3, 4)

WEIGHT_NAMES = (
    "conv_in_w", "conv_in_b", "gates_w", "gates_b", "conv_out_w",
    "conv_out_b", "w0", "b0", "w1", "b1", "wc", "bc", "wa", "ba",
)


def _conv2d(x, w, b, pad):
    y = jax.lax.conv_general_dilated(
        x, w, window_strides=(1, 1), padding=[(pad, pad), (pad, pad)],
        dimension_numbers=("NCHW", "OIHW", "NCHW"))
    return y + b[None, :, None, None]


def _cell(x, h, c, w, b):
    gates = _conv2d(jnp.concatenate([x, h], axis=1), w, b, pad=1)
    i, r, o, g = jnp.split(gates, 4, axis=1)
    i, r, o = jax.nn.sigmoid(i), jax.nn.sigmoid(r), jax.nn.sigmoid(o)
    g = jnp.tanh(g)
    c_new = r * c + i * g
    h_new = o * jnp.tanh(c_new)
    return h_new, c_new


def _forward(x, hx, cx, conv_in_w, conv_in_b, gates_w, gates_b,
             conv_out_w, conv_out_b, w0, b0, w1, b1, wc, bc, wa, ba):
    h = jax.nn.relu(_conv2d(x, conv_in_w, conv_in_b, pad=0))
    hs, cs = [], []
    cur = h
    for k in range(5):
        idx = CELL_ORDER[k]
        cur, c_new = _cell(cur, hx[idx], cx[idx], gates_w[k], gates_b[k])
        hs.append(cur)
        cs.append(c_new)
    y = jnp.tanh(_conv2d(cur, conv_out_w, conv_out_b, pad=1))
    y = y.reshape(y.shape[0], -1)
    y = jnp.tanh(y @ w0.T + b0)
    y = jnp.tanh(y @ w1.T + b1)
    value = y @ wc.T + bc
    action = y @ wa.T + ba
    return value, action, jnp.stack(hs), jnp.stack(cs)


_PMAPPED = jax.pmap(
    _forward,
    axis_name="core",
    # Map x over axis 0, hx/cx over axis 1 (their batch axis, pre-split),
    # replicate weights. hs/cs come back with the device axis at position 1
    # so host-side reassembly is a zero-copy reshape, not a transpose.
    in_axes=(0, 1, 1) + (None,) * len(WEIGHT_NAMES),
    out_axes=(0, 0, 1, 1),
    devices=jax.devices()[:NCORES],
)


def kernel(**inputs):
    x = np.asarray(inputs["x"])
    hx = np.asarray(inputs["hx"])
    cx = np.asarray(inputs["cx"])

    # Views only — no host copies.
    xs = x.reshape(NCORES, BL, C_IN, H, W)
    hxs = hx.reshape(5, NCORES, BL, NH, H, W)
    cxs = cx.reshape(5, NCORES, BL, NH, H, W)

    weights = [np.asarray(inputs[n]) for n in WEIGHT_NAMES]

    value, action, hs, cs = _PMAPPED(xs, hxs, cxs, *weights)

    value = np.asarray(value).reshape(B, 1)
    action = np.asarray(action).reshape(B, 800)
    hs = np.asarray(hs).reshape(5, B, NH, H, W)
    cs = np.asarray(cs).reshape(5, B, NH, H, W)
    return (value.astype(np.float32), action.astype(np.float32),
            hs.astype(np.float32), cs.astype(np.float32))
